# revision 17
# baseline (speedup 1.0000x reference)
"""CP-decomposed 3D conv (AirConv3D) on 8 Trainium2 NeuronCores.

Math (reference):
  out[o,X,Y,Z] = sum_{i,j,l,c,r} xp[c,X+i,Y+j,Z+l] * U_kh[i,r]*U_kw[j,r]*U_kd[l,r]
                  * U_cin[c,r]*U_cout[r,o] + bias[o]
  xp = zero-padded x (pad 1), kernel 3x3x3, CP rank 53.

Device pipeline per core (H-sharded, 7 output rows + 1-row halo each side).
The y-tap conv is split between engines to balance the pipeline:
  yh0 (out y 0..27):  folded into S1 as 3 PSUM-accumulated K=96 matmuls
      (weights w96j = U_kh (x) U_cin * U_kw[j], rhs read at y+j); ACT evicts
      PSUM -> bf16 t3 rows 0..27 directly.
  yh1 (out y 28..55): single K=96 matmul (plain U_kh (x) U_cin weights) over
      30 y rows -> t2; ACT evicts; DVE computes the 3-tap y-conv
      (3 tensor_scalar + 2 tensor_tensor) into t3 rows 28..55.
  S5: 3 accumulated z-shifted matmuls (K=53 per z-half, 2-way zh column-group
      concurrency); yc pairs share a 2-bank PSUM tile so E2 retires each
      half-row in one ACT op with fused per-partition bias.
  DMA: input chunks and output rows both on the SP ring (ACT stays off the
      DMA-issue path); host upcasts bf16.
Emission is software-pipelined 2 rows ahead.
"""

import os
import numpy as np
import ml_dtypes

import concourse.bass as bass
import concourse.bacc as bacc
import concourse.mybir as mybir
import concourse.tile as tile
from concourse.bass_utils import run_bass_kernel_spmd

BF16 = ml_dtypes.bfloat16

CIN, COUT, R, KK = 32, 64, 53, 3
H = W = D = 56
HP = 58            # padded spatial
NCORES = 8
XO = 7             # output H-rows per core
XR = 9             # input H-rows per core (with halo)
Z30 = 30           # z-window per z-half (28 out + 2 halo)
PLANE = HP * HP    # 3364
FD_X = XR * PLANE  # 30276
FD_T2 = 30 * Z30   # unfolded-half t2: 30 y rows x 30 z  (900)
FD_T3 = H * Z30    # 1680
FD_OX = H * 28     # 1568

_cache = {}


def _build_program():
    nc = bacc.Bacc("TRN2", debug=False, num_devices=NCORES)
    f32, bf16 = mybir.dt.float32, mybir.dt.bfloat16

    # pre-packed input: 3 H-shifted copies on partition blocks of 32
    x3_d = nc.dram_tensor("x3", [96, FD_X], mybir.dt.float8e4, kind="ExternalInput").ap()
    # packed weights: S5 z-tap blocks, S1 y-tap + plain blocks, scalars
    wa_d = nc.dram_tensor("wa", [128, 192], bf16, kind="ExternalInput").ap()
    wj_d = nc.dram_tensor("wj", [96, 256], bf16, kind="ExternalInput").ap()
    wf_d = nc.dram_tensor("wf", [128, 4], f32, kind="ExternalInput").ap()
    # [zh, o, x, y, z28]: matches SBUF partition-major layout; host reassembles
    out_d = nc.dram_tensor("out", [2, COUT, XO, H, 28], bf16,
                           kind="ExternalOutput").ap()

    with tile.TileContext(nc) as tc:
        with (
            tc.tile_pool(name="const", bufs=1) as cpool,
            tc.tile_pool(name="work", bufs=3) as wpool,
            tc.tile_pool(name="outp", bufs=3) as opool,
            tc.tile_pool(name="ps1", bufs=2, space="PSUM") as ps1,
            tc.tile_pool(name="ps2", bufs=2, space="PSUM") as ps2,
        ):
            X3 = cpool.tile([96, FD_X], mybir.dt.float8e4)
            WA = cpool.tile([128, 192], bf16)   # wa0 | wa1 | wa2
            WJ = cpool.tile([96, 256], bf16)    # w96j0 | j1 | j2 | plain
            WF = cpool.tile([128, 4], f32)      # s0 s1 s2 biasT

            # weights via the ACT HWDGE ring; input chunks own the SP ring
            nc.scalar.dma_start(WA[:, :], wa_d[:, :])
            nc.scalar.dma_start(WJ[:, :], wj_d[:, :])
            nc.scalar.dma_start(WF[:, :], wf_d[:, :])
            was = [WA[0:128, 64 * l:64 * (l + 1)] for l in range(KK)]
            w96j = [WJ[0:96, 64 * j:64 * (j + 1)] for j in range(KK)]
            w96p = WJ[0:96, 192:256]
            ss = [WF[0:128, j:j + 1] for j in range(KK)]
            biasT = WF[0:128, 3:4]

            # chunked input DMAs (one per H-row plane); trim zero tails
            nc.sync.dma_start(X3[0:96, 0:30 * HP], x3_d[0:96, 0:30 * HP])
            nc.sync.dma_start(X3[0:96, 30 * HP:PLANE], x3_d[0:96, 30 * HP:PLANE])
            for r in range(1, XR):
                pmax = 96 if r < XO else (64 if r == XO else 32)
                nc.sync.dma_start(X3[0:pmax, r * PLANE:(r + 1) * PLANE],
                                  x3_d[0:pmax, r * PLANE:(r + 1) * PLANE])

            X3v = X3.rearrange("p (x y z) -> p x y z", x=XR, y=HP, z=HP)

            t2s, t3s = {}, {}

            def emit_s1(x):
                t3 = wpool.tile([128, FD_T3], bf16, name="t3", tag="t3", bufs=3)
                t3s[x] = t3
                # folded half (out y 0..27): 3 accumulated y-tap matmuls
                p1 = ps1.tile([128, 1024], f32, name="p1", tag="p1")
                for zc in range(2):   # z half-chunks of 15
                    for j in range(KK):
                        for zh in range(2):
                            nc.tensor.matmul(
                                p1[zh * 64:zh * 64 + 64,
                                   zc * 512:zc * 512 + 28 * 15],
                                w96j[j],
                                X3v[0:96, x, j:j + 28,
                                    zh * 28 + zc * 15:zh * 28 + zc * 15 + 15],
                                start=(j == 0), stop=(j == KK - 1),
                            )
                src = p1.rearrange("p (zc w) -> p zc w", zc=2)[
                    0:128, 0:2, 0:28 * 15].rearrange(
                    "p zc (y z) -> p zc y z", y=28)
                dst = t3.rearrange("p (y zc z) -> p zc y z", y=H, zc=2)[
                    0:128, 0:2, 0:28, 0:15]
                nc.scalar.activation(
                    dst, src, mybir.ActivationFunctionType.Copy)

                # unfolded half: plain matmul over input y rows 28..57
                t2 = wpool.tile([128, FD_T2], bf16, name="t2", tag="t2", bufs=3)
                t2s[x] = t2
                p1u = ps1.tile([128, 1024], f32, name="p1u", tag="p1")
                for zc in range(2):
                    for zh in range(2):
                        nc.tensor.matmul(
                            p1u[zh * 64:zh * 64 + 64,
                                zc * 512:zc * 512 + 30 * 15],
                            w96p,
                            X3v[0:96, x, 28:58,
                                zh * 28 + zc * 15:zh * 28 + zc * 15 + 15],
                            start=True, stop=True,
                        )
                src = p1u.rearrange("p (zc w) -> p zc w", zc=2)[
                    0:128, 0:2, 0:30 * 15].rearrange(
                    "p zc (y z) -> p zc y z", y=30)
                dst = t2.rearrange("p (y zc z) -> p zc y z", y=30, zc=2)[
                    0:128, 0:2, 0:30, 0:15]
                nc.scalar.activation(
                    dst, src, mybir.ActivationFunctionType.Copy)

            def emit_yc(x):
                # y-conv for the unfolded half -> t3 rows y 28..55
                t2 = t2s.pop(x)
                t3 = t3s[x]
                W840 = 28 * Z30
                t3h = t3[0:128, 28 * Z30:FD_T3]          # [128, 840]
                t3b = wpool.tile([128, W840], bf16, name="t3b", tag="t3b", bufs=2)
                t3c = wpool.tile([128, W840], bf16, name="t3c", tag="t3c", bufs=2)
                nc.vector.tensor_scalar_mul(
                    t3b[0:128, :], t2[0:128, 0:W840], ss[0])
                nc.vector.tensor_scalar_mul(
                    t3c[0:128, :], t2[0:128, Z30:Z30 + W840], ss[1])
                nc.vector.tensor_tensor(
                    t3b[0:128, :], t3b[0:128, :], t3c[0:128, :],
                    mybir.AluOpType.add)
                nc.vector.tensor_scalar_mul(
                    t3c[0:128, :], t2[0:128, 2 * Z30:2 * Z30 + W840], ss[2])
                nc.vector.tensor_tensor(
                    t3h, t3b[0:128, :], t3c[0:128, :], mybir.AluOpType.add)

            def emit_s5(x):
                # S5 + paired E2 (ACT, fused bias) + out-DMA on the SP ring
                t3 = t3s.pop(x)
                t3v = t3.rearrange("p (y z) -> p y z", y=H, z=Z30)
                ox = opool.tile([128, FD_OX], mybir.dt.bfloat16, name="ox", tag="ox")
                for h in range(2):       # half-rows: yc pairs share a psum tile
                    p2 = ps2.tile([128, 1024], f32, name="p2", tag="p2")
                    for ycs in range(2):
                        yc = 2 * h + ycs
                        yb = yc * 14
                        c0 = ycs * 512
                        for l in range(KK):
                            for zh in range(2):
                                nc.tensor.matmul(
                                    p2[zh * 64:zh * 64 + 64, c0:c0 + 392],
                                    was[l][zh * 64:zh * 64 + 53, 0:64],
                                    t3v[zh * 64:zh * 64 + 53, yb:yb + 14, l:l + 28],
                                    start=(l == 0), stop=(l == KK - 1),
                                )
                    nc.scalar.activation(
                        ox[0:128, h * 784:h * 784 + 784].rearrange(
                            "p (c w) -> p c w", c=2),
                        p2.rearrange("p (c w) -> p c w", c=2)[0:128, 0:2, 0:392],
                        mybir.ActivationFunctionType.Identity,
                        bias=biasT,
                    )
                if x == XO - 1:
                    nc.sync.dma_start(out_d[0:2, 0:COUT, x, 0:28, 0:28],
                                      ox[0:128, 0:FD_OX // 2])
                    nc.sync.dma_start(out_d[0:2, 0:COUT, x, 28:H, 0:28],
                                      ox[0:128, FD_OX // 2:FD_OX])
                else:
                    nc.sync.dma_start(out_d[0:2, 0:COUT, x, 0:H, 0:28],
                                      ox[0:128, 0:FD_OX])

            # software-pipelined emission (2 rows ahead)
            emit_s1(0)
            emit_yc(0)
            emit_s1(1)
            emit_yc(1)
            for x in range(XO):
                emit_s5(x)
                if x + 2 < XO:
                    emit_s1(x + 2)
                    emit_yc(x + 2)

    nc.compile()
    return nc


def _prep_weights(U_kh, U_kw, U_kd, U_cin, U_cout, bias):
    U_kh, U_kw, U_kd = (np.asarray(a, np.float32) for a in (U_kh, U_kw, U_kd))
    U_cin, U_cout = np.asarray(U_cin, np.float32), np.asarray(U_cout, np.float32)
    bias = np.asarray(bias, np.float32)

    wa = np.zeros((128, 192), np.float32)
    for l in range(KK):
        v = U_kd[l][:, None] * U_cout
        wa[0:R, 64 * l:64 * (l + 1)] = v
        wa[64:64 + R, 64 * l:64 * (l + 1)] = v

    w96 = (U_kh[:, None, :] * U_cin[None, :, :]).reshape(96, R)
    wj = np.zeros((96, 256), np.float32)
    for j in range(KK):
        wj[:, 64 * j:64 * j + R] = w96 * U_kw[j][None, :]
    wj[:, 192:192 + R] = w96

    wf = np.zeros((128, 4), np.float32)
    for j in range(KK):
        wf[0:R, j] = U_kw[j]
        wf[64:64 + R, j] = U_kw[j]
    wf[0:64, 3] = bias
    wf[64:128, 3] = bias
    return wa.astype(BF16), wj.astype(BF16), wf


def kernel(x, U_kh, U_kw, U_kd, U_cin, U_cout, bias):
    x = np.asarray(x, np.float32)
    assert x.shape == (1, CIN, H, W, D)

    if "nc" not in _cache:
        _cache["nc"] = _build_program()
    nc = _cache["nc"]

    wa, wj, wf = _prep_weights(U_kh, U_kw, U_kd, U_cin, U_cout, bias)

    xp = np.zeros((CIN, HP, HP, HP), np.float32)
    xp[:, 1:57, 1:57, 1:57] = x[0]
    xp = xp.astype(ml_dtypes.float8_e4m3)

    in_maps = []
    for k in range(NCORES):
        shard = xp[:, 7 * k:7 * k + XR].reshape(CIN, FD_X)
        x3 = np.zeros((96, FD_X), ml_dtypes.float8_e4m3)
        x3[0:32] = shard
        x3[32:64, 0:FD_X - PLANE] = shard[:, PLANE:]
        x3[64:96, 0:FD_X - 2 * PLANE] = shard[:, 2 * PLANE:]
        in_maps.append({"x3": x3, "wa": wa, "wj": wj, "wf": wf})

    trace = bool(int(os.environ.get("KERNEL_PROFILE", "0")))
    res = run_bass_kernel_spmd(nc, in_maps, core_ids=list(range(NCORES)),
                               trace=trace)
    if trace and res.exec_time_ns is not None:
        print(f"HW exec time: {res.exec_time_ns} ns")
        _cache["exec_time_ns"] = res.exec_time_ns

    out = np.empty((1, COUT, H, W, D), np.float32)
    for k in range(NCORES):
        r = np.asarray(res.results[k]["out"], np.float32).reshape(
            2, COUT, XO, H, 28)
        out[0, :, 7 * k:7 * k + XO] = r.transpose(1, 2, 3, 0, 4).reshape(
            COUT, XO, H, D)
    return out


if __name__ == "__main__":
    rng = np.random.default_rng(0)
    ins = {
        "x": rng.standard_normal((1, CIN, H, W, D)).astype(np.float32),
        "U_kh": (rng.standard_normal((KK, R)) * 0.1).astype(np.float32),
        "U_kw": (rng.standard_normal((KK, R)) * 0.1).astype(np.float32),
        "U_kd": (rng.standard_normal((KK, R)) * 0.1).astype(np.float32),
        "U_cin": (rng.standard_normal((CIN, R)) * 0.1).astype(np.float32),
        "U_cout": (rng.standard_normal((R, COUT)) * 0.1).astype(np.float32),
        "bias": rng.standard_normal((CIN * 2,))[:COUT].astype(np.float32),
    }
    o = kernel(**ins)
    print("kernel ran, out shape", o.shape, "mean", float(np.abs(o).mean()))


# revision 21
# speedup vs baseline: 1.0023x; 1.0023x over previous
"""CP-decomposed 3D conv (AirConv3D) on 8 Trainium2 NeuronCores.

Math (reference):
  out[o,X,Y,Z] = sum_{i,j,l,c,r} xp[c,X+i,Y+j,Z+l] * U_kh[i,r]*U_kw[j,r]*U_kd[l,r]
                  * U_cin[c,r]*U_cout[r,o] + bias[o]
  xp = zero-padded x (pad 1), kernel 3x3x3, CP rank 53.

Device pipeline per core (H-sharded, 7 output rows + 1-row halo each side).
The y-tap conv is split between engines to balance the pipeline:
  yh0 (out y 0..27):  folded into S1 as 3 PSUM-accumulated K=96 matmuls
      (weights w96j = U_kh (x) U_cin * U_kw[j], rhs read at y+j); ACT evicts
      PSUM -> bf16 t3 rows 0..27 directly.
  yh1 (out y 28..55): single K=96 matmul (plain U_kh (x) U_cin weights) over
      30 y rows -> t2; ACT evicts; DVE computes the 3-tap y-conv
      (3 tensor_scalar + 2 tensor_tensor) into t3 rows 28..55.
  S5: 3 accumulated z-shifted matmuls (K=53 per z-half, 2-way zh column-group
      concurrency); yc pairs share a 2-bank PSUM tile so E2 retires each
      half-row in one ACT op with fused per-partition bias.
  DMA: input chunks and output rows both on the SP ring (ACT stays off the
      DMA-issue path); host upcasts bf16.
Emission is software-pipelined 2 rows ahead.
"""

import os
import numpy as np
import ml_dtypes

import concourse.bass as bass
import concourse.bacc as bacc
import concourse.mybir as mybir
import concourse.tile as tile
from concourse.bass_utils import run_bass_kernel_spmd

BF16 = ml_dtypes.bfloat16

CIN, COUT, R, KK = 32, 64, 53, 3
H = W = D = 56
HP = 58            # padded spatial
NCORES = 8
XO = 7             # output H-rows per core
XR = 9             # input H-rows per core (with halo)
Z30 = 30           # z-window per z-half (28 out + 2 halo)
PLANE = HP * HP    # 3364
FD_X = XR * PLANE  # 30276
FD_T2 = 30 * Z30   # unfolded-half t2: 30 y rows x 30 z  (900)
FD_T3 = H * Z30    # 1680
FD_OX = H * 28     # 1568

_cache = {}


def _build_program():
    nc = bacc.Bacc("TRN2", debug=False, num_devices=NCORES)
    f32, bf16 = mybir.dt.float32, mybir.dt.bfloat16

    # pre-packed input: 3 H-shifted copies on partition blocks of 32,
    # plane-major in DRAM so each plane DMA is one contiguous block
    x3_d = nc.dram_tensor("x3", [XR, 96, PLANE], mybir.dt.float8e4,
                          kind="ExternalInput").ap()
    # packed weights: S5 z-tap blocks, S1 y-tap + plain blocks, scalars
    wa_d = nc.dram_tensor("wa", [128, 192], bf16, kind="ExternalInput").ap()
    wj_d = nc.dram_tensor("wj", [96, 256], bf16, kind="ExternalInput").ap()
    wf_d = nc.dram_tensor("wf", [128, 4], f32, kind="ExternalInput").ap()
    # [zh, o, x, y, z28]: matches SBUF partition-major layout; host reassembles
    out_d = nc.dram_tensor("out", [2, COUT, XO, H, 28], bf16,
                           kind="ExternalOutput").ap()

    with tile.TileContext(nc) as tc:
        with (
            tc.tile_pool(name="const", bufs=1) as cpool,
            tc.tile_pool(name="work", bufs=3) as wpool,
            tc.tile_pool(name="outp", bufs=3) as opool,
            tc.tile_pool(name="ps1", bufs=2, space="PSUM") as ps1,
            tc.tile_pool(name="ps2", bufs=2, space="PSUM") as ps2,
        ):
            X3 = cpool.tile([96, FD_X], mybir.dt.float8e4)
            WA = cpool.tile([128, 192], bf16)   # wa0 | wa1 | wa2
            WJ = cpool.tile([96, 256], bf16)    # w96j0 | j1 | j2 | plain
            WF = cpool.tile([128, 4], f32)      # s0 s1 s2 biasT

            # weights via the ACT HWDGE ring; input chunks own the SP ring
            nc.scalar.dma_start(WA[:, :], wa_d[:, :])
            nc.scalar.dma_start(WJ[:, :], wj_d[:, :])
            nc.scalar.dma_start(WF[:, :], wf_d[:, :])
            was = [WA[0:128, 64 * l:64 * (l + 1)] for l in range(KK)]
            w96j = [WJ[0:96, 64 * j:64 * (j + 1)] for j in range(KK)]
            w96p = WJ[0:96, 192:256]
            ss = [WF[0:128, j:j + 1] for j in range(KK)]
            biasT = WF[0:128, 3:4]

            # chunked input DMAs (one per H-row plane); trim zero tails
            nc.sync.dma_start(X3[0:96, 0:30 * HP], x3_d[0, 0:96, 0:30 * HP])
            nc.sync.dma_start(X3[0:96, 30 * HP:PLANE], x3_d[0, 0:96, 30 * HP:PLANE])
            for r in range(1, XR):
                pmax = 96 if r < XO else (64 if r == XO else 32)
                nc.sync.dma_start(X3[0:pmax, r * PLANE:(r + 1) * PLANE],
                                  x3_d[r, 0:pmax, 0:PLANE])

            X3v = X3.rearrange("p (x y z) -> p x y z", x=XR, y=HP, z=HP)

            # PE warmup: dummy matmuls on a zeroed scratch tile keep the PE
            # busy through the input-DMA prologue so the 2.4 GHz p-state ramp
            # is underway before row 0 arrives.
            SCR = cpool.tile([96, 512], bf16)
            nc.vector.memset(SCR[:, :], 0.0)
            pw = ps1.tile([128, 1024], f32, name="pw", tag="p1")
            for i in range(8):
                for zh in range(2):
                    nc.tensor.matmul(
                        pw[zh * 64:zh * 64 + 64, 0:420],
                        SCR[0:96, 0:64], SCR[0:96, 64:484],
                        start=True, stop=True)

            t2s, t3s = {}, {}

            def emit_s1(x):
                t3 = wpool.tile([128, FD_T3], bf16, name="t3", tag="t3", bufs=3)
                t3s[x] = t3
                # folded half (out y 0..27): 3 accumulated y-tap matmuls
                p1 = ps1.tile([128, 1024], f32, name="p1", tag="p1")
                for zc in range(2):   # z half-chunks of 15
                    for j in range(KK):
                        for zh in range(2):
                            nc.tensor.matmul(
                                p1[zh * 64:zh * 64 + 64,
                                   zc * 512:zc * 512 + 28 * 15],
                                w96j[j],
                                X3v[0:96, x, j:j + 28,
                                    zh * 28 + zc * 15:zh * 28 + zc * 15 + 15],
                                start=(j == 0), stop=(j == KK - 1),
                            )
                src = p1.rearrange("p (zc w) -> p zc w", zc=2)[
                    0:128, 0:2, 0:28 * 15].rearrange(
                    "p zc (y z) -> p zc y z", y=28)
                dst = t3.rearrange("p (y zc z) -> p zc y z", y=H, zc=2)[
                    0:128, 0:2, 0:28, 0:15]
                nc.scalar.activation(
                    dst, src, mybir.ActivationFunctionType.Copy)

                # unfolded half: plain matmul over input y rows 28..57
                t2 = wpool.tile([128, FD_T2], bf16, name="t2", tag="t2", bufs=3)
                t2s[x] = t2
                p1u = ps1.tile([128, 1024], f32, name="p1u", tag="p1")
                for zc in range(2):
                    for zh in range(2):
                        nc.tensor.matmul(
                            p1u[zh * 64:zh * 64 + 64,
                                zc * 512:zc * 512 + 30 * 15],
                            w96p,
                            X3v[0:96, x, 28:58,
                                zh * 28 + zc * 15:zh * 28 + zc * 15 + 15],
                            start=True, stop=True,
                        )
                src = p1u.rearrange("p (zc w) -> p zc w", zc=2)[
                    0:128, 0:2, 0:30 * 15].rearrange(
                    "p zc (y z) -> p zc y z", y=30)
                dst = t2.rearrange("p (y zc z) -> p zc y z", y=30, zc=2)[
                    0:128, 0:2, 0:30, 0:15]
                nc.scalar.activation(
                    dst, src, mybir.ActivationFunctionType.Copy)

            def emit_yc(x):
                # y-conv for the unfolded half -> t3 rows y 28..55
                t2 = t2s.pop(x)
                t3 = t3s[x]
                W840 = 28 * Z30
                t3h = t3[0:128, 28 * Z30:FD_T3]          # [128, 840]
                t3b = wpool.tile([128, W840], bf16, name="t3b", tag="t3b", bufs=3)
                t3c = wpool.tile([128, W840], bf16, name="t3c", tag="t3c", bufs=3)
                nc.vector.tensor_scalar_mul(
                    t3b[0:128, :], t2[0:128, 0:W840], ss[0])
                nc.vector.tensor_scalar_mul(
                    t3c[0:128, :], t2[0:128, Z30:Z30 + W840], ss[1])
                nc.vector.tensor_tensor(
                    t3b[0:128, :], t3b[0:128, :], t3c[0:128, :],
                    mybir.AluOpType.add)
                nc.vector.tensor_scalar_mul(
                    t3c[0:128, :], t2[0:128, 2 * Z30:2 * Z30 + W840], ss[2])
                nc.vector.tensor_tensor(
                    t3h, t3b[0:128, :], t3c[0:128, :], mybir.AluOpType.add)

            def emit_s5(x):
                # S5 + paired E2 (ACT, fused bias) + out-DMA on the SP ring
                t3 = t3s.pop(x)
                t3v = t3.rearrange("p (y z) -> p y z", y=H, z=Z30)
                ox = opool.tile([128, FD_OX], mybir.dt.bfloat16, name="ox", tag="ox")
                for h in range(2):       # half-rows: yc pairs share a psum tile
                    p2 = ps2.tile([128, 1024], f32, name="p2", tag="p2")
                    for ycs in range(2):
                        yc = 2 * h + ycs
                        yb = yc * 14
                        c0 = ycs * 512
                        for l in range(KK):
                            for zh in range(2):
                                nc.tensor.matmul(
                                    p2[zh * 64:zh * 64 + 64, c0:c0 + 392],
                                    was[l][zh * 64:zh * 64 + 53, 0:64],
                                    t3v[zh * 64:zh * 64 + 53, yb:yb + 14, l:l + 28],
                                    start=(l == 0), stop=(l == KK - 1),
                                )
                    nc.scalar.activation(
                        ox[0:128, h * 784:h * 784 + 784].rearrange(
                            "p (c w) -> p c w", c=2),
                        p2.rearrange("p (c w) -> p c w", c=2)[0:128, 0:2, 0:392],
                        mybir.ActivationFunctionType.Identity,
                        bias=biasT,
                    )
                if x == XO - 1:
                    nc.sync.dma_start(out_d[0:2, 0:COUT, x, 0:28, 0:28],
                                      ox[0:128, 0:FD_OX // 2])
                    nc.sync.dma_start(out_d[0:2, 0:COUT, x, 28:H, 0:28],
                                      ox[0:128, FD_OX // 2:FD_OX])
                else:
                    nc.sync.dma_start(out_d[0:2, 0:COUT, x, 0:H, 0:28],
                                      ox[0:128, 0:FD_OX])

            # software-pipelined emission (2 rows ahead)
            emit_s1(0)
            emit_yc(0)
            emit_s1(1)
            emit_yc(1)
            for x in range(XO):
                emit_s5(x)
                if x + 2 < XO:
                    emit_s1(x + 2)
                    emit_yc(x + 2)

    nc.compile()
    return nc


def _prep_weights(U_kh, U_kw, U_kd, U_cin, U_cout, bias):
    U_kh, U_kw, U_kd = (np.asarray(a, np.float32) for a in (U_kh, U_kw, U_kd))
    U_cin, U_cout = np.asarray(U_cin, np.float32), np.asarray(U_cout, np.float32)
    bias = np.asarray(bias, np.float32)

    wa = np.zeros((128, 192), np.float32)
    for l in range(KK):
        v = U_kd[l][:, None] * U_cout
        wa[0:R, 64 * l:64 * (l + 1)] = v
        wa[64:64 + R, 64 * l:64 * (l + 1)] = v

    w96 = (U_kh[:, None, :] * U_cin[None, :, :]).reshape(96, R)
    wj = np.zeros((96, 256), np.float32)
    for j in range(KK):
        wj[:, 64 * j:64 * j + R] = w96 * U_kw[j][None, :]
    wj[:, 192:192 + R] = w96

    wf = np.zeros((128, 4), np.float32)
    for j in range(KK):
        wf[0:R, j] = U_kw[j]
        wf[64:64 + R, j] = U_kw[j]
    wf[0:64, 3] = bias
    wf[64:128, 3] = bias
    return wa.astype(BF16), wj.astype(BF16), wf


def kernel(x, U_kh, U_kw, U_kd, U_cin, U_cout, bias):
    x = np.asarray(x, np.float32)
    assert x.shape == (1, CIN, H, W, D)

    if "nc" not in _cache:
        _cache["nc"] = _build_program()
    nc = _cache["nc"]

    wa, wj, wf = _prep_weights(U_kh, U_kw, U_kd, U_cin, U_cout, bias)

    xp = np.zeros((CIN, HP, HP, HP), np.float32)
    xp[:, 1:57, 1:57, 1:57] = x[0]
    xp = xp.astype(ml_dtypes.float8_e4m3)

    in_maps = []
    for k in range(NCORES):
        shard = xp[:, 7 * k:7 * k + XR]          # [32, XR, HP, HP]
        x3 = np.zeros((XR, 96, PLANE), ml_dtypes.float8_e4m3)
        for r in range(XR):
            x3[r, 0:32] = shard[:, r].reshape(CIN, PLANE)
            if r + 1 < XR:
                x3[r, 32:64] = shard[:, r + 1].reshape(CIN, PLANE)
            if r + 2 < XR:
                x3[r, 64:96] = shard[:, r + 2].reshape(CIN, PLANE)
        in_maps.append({"x3": x3, "wa": wa, "wj": wj, "wf": wf})

    trace = bool(int(os.environ.get("KERNEL_PROFILE", "0")))
    res = run_bass_kernel_spmd(nc, in_maps, core_ids=list(range(NCORES)),
                               trace=trace)
    if trace and res.exec_time_ns is not None:
        print(f"HW exec time: {res.exec_time_ns} ns")
        _cache["exec_time_ns"] = res.exec_time_ns

    out = np.empty((1, COUT, H, W, D), np.float32)
    for k in range(NCORES):
        r = np.asarray(res.results[k]["out"], np.float32).reshape(
            2, COUT, XO, H, 28)
        out[0, :, 7 * k:7 * k + XO] = r.transpose(1, 2, 3, 0, 4).reshape(
            COUT, XO, H, D)
    return out


if __name__ == "__main__":
    rng = np.random.default_rng(0)
    ins = {
        "x": rng.standard_normal((1, CIN, H, W, D)).astype(np.float32),
        "U_kh": (rng.standard_normal((KK, R)) * 0.1).astype(np.float32),
        "U_kw": (rng.standard_normal((KK, R)) * 0.1).astype(np.float32),
        "U_kd": (rng.standard_normal((KK, R)) * 0.1).astype(np.float32),
        "U_cin": (rng.standard_normal((CIN, R)) * 0.1).astype(np.float32),
        "U_cout": (rng.standard_normal((R, COUT)) * 0.1).astype(np.float32),
        "bias": rng.standard_normal((CIN * 2,))[:COUT].astype(np.float32),
    }
    o = kernel(**ins)
    print("kernel ran, out shape", o.shape, "mean", float(np.abs(o).mean()))
